# revision 1
# baseline (speedup 1.0000x reference)
"""AnchorTargetLayer max-IoU kernel for 8 TRN2 NeuronCores.

max_iou[b, n] = max_g IoU(anchor_n, gt_box[b, g]);
anchors [100000, 4] f32, gt_boxes [4, 64, 4] f32 -> out [4, 100000] f32.

Sharding: anchors split 8 ways (12544/core incl pad), gt replicated,
no collectives. Per-core layout: anchors on SBUF partitions (128 per
block, 98 blocks), all B*G = 256 (batch, gt) pairs on the free dim.
GT-derived rows are broadcast once into [128, 5*256] SBUF (loop
invariant); per-anchor coords are [128,1] per-partition scalars.

Math per (anchor, pair):
  iw   = min(ax2, gx2) + min(-ax1, -gx1)        (x-overlap, may be <0)
  ih   = likewise in y
  inter = relu(iw) * relu(ih)
  v    = (areaA + areaG) * (1/inter)            (= union/inter + 1)
  vmin[b] = min_g v ;  out = 1/(min(vmin,1e30) - 1)
(inter == 0 -> Reciprocal gives inf -> dropped by the min; an anchor
with no overlap at all ends at exactly 0 via the 1e30 clamp.)

Engine split, software-pipelined with per-instruction semaphore ticks:
  Vector (DVE):  tx, sx, ty, sy (the two 1-D overlap chains),
                 inter = relu(sx)*ihr, and the per-batch min reduce
  Scalar (ACT):  ihr = Relu(sy), SA = areaA+areaG (Identity+bias),
                 rint = Reciprocal(inter)  [all share one table set]
  GpSimd (Pool): v = SA * rint  (tensor_tensor mult)
DVE is the bottleneck (~90% busy); ACT ~60%, Pool ~35%.
"""

import os
import sys

import numpy as np

sys.path.insert(0, "/opt/trn_rl_repo")

import concourse.bass as bass
import concourse.mybir as mybir
from concourse.bass_utils import run_bass_kernel_spmd

N_ANCHORS = 100000
BATCH = 4
N_GT = 64
N_CORES = 8

P = 128
BLOCKS = 98
N_LOC = P * BLOCKS          # 12544
N_PAD = N_LOC * N_CORES     # 100352
NPAIR = BATCH * N_GT        # 256

F32 = mybir.dt.float32
EPS = 1e-15

LAST_EXEC_NS = None


def _ensure_axon_ntff_hook():
    try:
        import antenv.axon_hooks  # noqa: F401

        return
    except ImportError:
        pass
    import contextlib
    import ctypes
    import types

    import antenv

    m = types.ModuleType("antenv.axon_hooks")
    m._hook = None

    def set_axon_ntff_profile_hook(h):
        m._hook = h

    def get_axon_ntff_profile_hook():
        return m._hook

    m.set_axon_ntff_profile_hook = set_axon_ntff_profile_hook
    m.get_axon_ntff_profile_hook = get_axon_ntff_profile_hook
    sys.modules["antenv.axon_hooks"] = m
    antenv.axon_hooks = m

    so_path = os.environ.get("PJRT_LIBRARY_PATH", "/opt/axon/libaxon_pjrt.so")
    try:
        lib = ctypes.CDLL(so_path)
    except OSError:
        return
    if not hasattr(lib, "axon_start_nrt_profile"):
        return
    lib.axon_start_nrt_profile.argtypes = [
        ctypes.POINTER(ctypes.c_int64),
        ctypes.c_size_t,
    ]
    lib.axon_start_nrt_profile.restype = ctypes.c_int64
    lib.axon_stop_nrt_profile.argtypes = [ctypes.c_char_p]
    lib.axon_stop_nrt_profile.restype = ctypes.c_int64

    @contextlib.contextmanager
    def _hook(output_dir, device_ids):
        import jax

        jax.devices()
        if device_ids:
            ids = (ctypes.c_int64 * len(device_ids))(*device_ids)
            rc = lib.axon_start_nrt_profile(ids, len(device_ids))
        else:
            rc = lib.axon_start_nrt_profile(None, 0)
        if rc != 0:
            raise RuntimeError(f"axon_start_nrt_profile rc={rc}")
        try:
            yield
        finally:
            n = lib.axon_stop_nrt_profile(str(output_dir).encode())
            if n < 0:
                raise RuntimeError(f"axon_stop_nrt_profile rc={n}")

    set_axon_ntff_profile_hook(_hook)


def _patch_upload_artifacts():
    import concourse.bass_utils as bu

    if getattr(bu.upload_artifacts, "_safe", False):
        return
    orig = bu.upload_artifacts

    def safe(tmpdir):
        try:
            return orig(tmpdir)
        except Exception:
            return tmpdir

    safe._safe = True
    bu.upload_artifacts = safe


def _act_recip(scalar_eng, nc, out_ap, in_ap):
    """Directly emit Activation(Reciprocal) (the nc.scalar.activation wrapper
    rejects Reciprocal)."""
    ins = [scalar_eng.lower_ap(in_ap)]
    for argv in (0.0, 1.0, 0.0):  # bias, scale, alpha
        ins.append(mybir.ImmediateValue(dtype=F32, value=argv))
    return scalar_eng.add_instruction(
        mybir.InstActivation(
            name=nc.get_next_instruction_name(),
            func=mybir.ActivationFunctionType.Reciprocal,
            ins=ins,
            outs=[scalar_eng.lower_ap(out_ap)],
        )
    )


class _Ticks:
    """Per-engine completion tick bookkeeping for cross-engine waits.

    Tick numbers are precomputed from the schedule (so any engine stream can
    be emitted first); each tracked instruction gets `.then_inc(sem, 1)`;
    waiters use wait_ge(sem, tick).
    """

    def __init__(self, orders, sems):
        # orders: {eng_name: [key, ...]} in emission order; sems: {eng: sem}
        self.tick_no = {}
        self.key_eng = {}
        for eng, keys in orders.items():
            for t, key in enumerate(keys, start=1):
                self.tick_no[key] = t
                self.key_eng[key] = eng
        self.sems = sems

    def mark(self, inst, key):
        inst.then_inc(self.sems[self.key_eng[key]], 1)

    def wait(self, engine, key):
        engine.wait_ge(self.sems[self.key_eng[key]], self.tick_no[key])


def _build_graph():
    nc = bass.Bass()
    A_ext = nc.declare_dram_parameter("anchors_p", [P, BLOCKS * 4], F32, isOutput=False)
    AR_ext = nc.declare_dram_parameter("aarea", [P, BLOCKS], F32, isOutput=False)
    GT_ext = nc.declare_dram_parameter("gtrows", [5, NPAIR], F32, isOutput=False)
    out_ext = nc.declare_dram_parameter("out", [P, BLOCKS * 4], F32, isOutput=True)

    Alu = mybir.AluOpType
    NB = 3  # cross-engine buffer depth

    with (
        nc.sbuf_tensor("A", [P, BLOCKS * 4], F32) as A,
        nc.sbuf_tensor("AR", [P, BLOCKS], F32) as AR,
        nc.sbuf_tensor("GTB", [P, 5, NPAIR], F32) as GTB,
        nc.sbuf_tensor("TX", [P, NPAIR], F32) as TX,
        nc.sbuf_tensor("TY", [P, NPAIR], F32) as TY,
        nc.sbuf_tensor("SX", [P, NB, NPAIR], F32) as SXb,
        nc.sbuf_tensor("SY", [P, NB, NPAIR], F32) as SYb,
        nc.sbuf_tensor("IHR", [P, NB, NPAIR], F32) as IHRb,
        nc.sbuf_tensor("INT", [P, NB, NPAIR], F32) as INTb,
        nc.sbuf_tensor("RI", [P, NB, NPAIR], F32) as RIb,
        nc.sbuf_tensor("SA", [P, NB, NPAIR], F32) as SAb,
        nc.sbuf_tensor("VB", [P, NB, NPAIR], F32) as VBb,
        nc.sbuf_tensor("MH", [P, NB, NPAIR // 2], F32) as MHb,
        nc.sbuf_tensor("VOUT", [P, BLOCKS * 4], F32) as VOUT,
        nc.sbuf_tensor("MIOU", [P, BLOCKS * 4], F32) as MIOU,
        nc.Block() as block,
        nc.semaphore("dma_sem") as dma_sem,
        nc.semaphore("dve_sem") as dve_sem,
        nc.semaphore("act_sem") as act_sem,
        nc.semaphore("pool_sem") as pool_sem,
    ):
        # ---- schedule (must mirror the emission loops below exactly) ----
        dve_order = []
        for s in range(BLOCKS + 3):
            if s < BLOCKS:
                dve_order.append(("sy", s))
            if s >= 1 and s - 1 < BLOCKS:
                dve_order.append(("inter", s - 1))
            if s >= 3 and s - 3 < BLOCKS:
                dve_order.append(("red", s - 3))
        dve_order.append(("vc", 0))
        act_order = []
        for s in range(BLOCKS + 2):
            if s >= 1 and s - 1 < BLOCKS:
                act_order.append(("ihr", s - 1))
                act_order.append(("sa", s - 1))
            if s >= 2 and s - 2 < BLOCKS:
                act_order.append(("rint", s - 2))
        act_order.append(("miou", 0))
        pool_order = []
        for s in range(BLOCKS + 2):
            if s >= 2 and s - 2 < BLOCKS:
                pool_order.append(("v", s - 2))

        tk = _Ticks(
            {"dve": dve_order, "act": act_order, "pool": pool_order},
            {"dve": dve_sem, "act": act_sem, "pool": pool_sem},
        )

        @block.sync
        def _(sync):
            sync.dma_start(out=A[:, :], in_=A_ext[:, :]).then_inc(dma_sem, 16)
            sync.dma_start(out=AR[:, :], in_=AR_ext[:, :]).then_inc(dma_sem, 16)
            g_ap = GT_ext[:, :]
            g_b = bass.AP(
                tensor=g_ap.tensor, offset=g_ap.offset, ap=[[0, P]] + list(g_ap.ap)
            )
            sync.dma_start(out=GTB[:, :, :], in_=g_b).then_inc(dma_sem, 16)

        GX1N = GTB[:, 0, :]
        GX2 = GTB[:, 1, :]
        GY1N = GTB[:, 2, :]
        GY2 = GTB[:, 3, :]
        GAREA = GTB[:, 4, :]

        # ---- emission: three engine streams, software-pipelined ----
        # stage offsets at "step" s (s = 0..BLOCKS+3):
        #   DVE: front(s)               [tx,sx,ty,sy]
        #   ACT: ihr(s-1), SA(s-1)
        #   DVE: inter(s-1)
        #   ACT: rint(s-2)
        #   Pool: v(s-2), mh(s-2)
        #   DVE: red(s-3)
        # Emission is per-engine (whole stream at once); waits use tick map.
        dve_prog = []
        act_prog = []
        pool_prog = []

        def dve_front(vector, j):
            nax1 = A[:, 4 * j + 0 : 4 * j + 1]
            ax2 = A[:, 4 * j + 1 : 4 * j + 2]
            nay1 = A[:, 4 * j + 2 : 4 * j + 3]
            ay2 = A[:, 4 * j + 3 : 4 * j + 4]
            b = j % NB
            if j >= NB:
                tk.wait(vector, ("ihr", j - NB))  # ACT done reading SX/SY slot
            vector.tensor_scalar(
                out=TX[:, :], in0=GX2, scalar1=ax2, scalar2=None, op0=Alu.min
            )
            vector.scalar_tensor_tensor(
                out=SXb[:, b, :], in0=GX1N, scalar=nax1, in1=TX[:, :],
                op0=Alu.min, op1=Alu.add,
            )
            vector.tensor_scalar(
                out=TY[:, :], in0=GY2, scalar1=ay2, scalar2=None, op0=Alu.min
            )
            i = vector.scalar_tensor_tensor(
                out=SYb[:, b, :], in0=GY1N, scalar=nay1, in1=TY[:, :],
                op0=Alu.min, op1=Alu.add,
            )
            tk.mark(i, ("sy", j))

        def dve_inter(vector, j):
            b = j % NB
            tk.wait(vector, ("ihr", j))
            if j >= NB:
                tk.wait(vector, ("rint", j - NB))  # ACT done reading INT slot
            i = vector.scalar_tensor_tensor(
                out=INTb[:, b, :], in0=SXb[:, b, :], scalar=0.0,
                in1=IHRb[:, b, :], op0=Alu.max, op1=Alu.mult,
            )
            tk.mark(i, ("inter", j))

        def dve_red(vector, j):
            b = j % NB
            tk.wait(vector, ("v", j))
            i = vector.tensor_reduce(
                out=VOUT[:, 4 * j : 4 * (j + 1)],
                in_=VBb[:, b, :].rearrange("p (bt g) -> p bt g", bt=BATCH),
                axis=mybir.AxisListType.X,
                op=Alu.min,
            )
            tk.mark(i, ("red", j))

        def act_mid(scalar, j):
            b = j % NB
            tk.wait(scalar, ("sy", j))
            if j >= NB:
                tk.wait(scalar, ("inter", j - NB))  # DVE done reading IHR slot
            i = scalar.activation(
                out=IHRb[:, b, :], in_=SYb[:, b, :],
                func=mybir.ActivationFunctionType.Relu,
            )
            tk.mark(i, ("ihr", j))
            if j >= NB:
                tk.wait(scalar, ("v", j - NB))  # Pool done reading SA slot
            i = scalar.activation(
                out=SAb[:, b, :], in_=GAREA,
                func=mybir.ActivationFunctionType.Identity,
                bias=AR[:, j : j + 1], scale=1.0,
            )
            tk.mark(i, ("sa", j))

        def act_rint(scalar, j):
            b = j % NB
            tk.wait(scalar, ("inter", j))
            if j >= NB:
                tk.wait(scalar, ("v", j - NB))  # Pool done reading RI slot
            i = _act_recip(scalar, nc, RIb[:, b, :], INTb[:, b, :])
            tk.mark(i, ("rint", j))

        def pool_v(gpsimd, j):
            b = j % NB
            tk.wait(gpsimd, ("rint", j))
            tk.wait(gpsimd, ("sa", j))
            if j >= NB:
                tk.wait(gpsimd, ("red", j - NB))  # DVE done reading VB slot
            i = gpsimd.tensor_tensor(
                out=VBb[:, b, :], in0=SAb[:, b, :], in1=RIb[:, b, :], op=Alu.mult
            )
            tk.mark(i, ("v", j))

        @block.vector
        def _(vector):
            vector.wait_ge(dma_sem, 48)
            for s in range(BLOCKS + 3):
                if s < BLOCKS:
                    dve_front(vector, s)
                if 1 <= s + 0 and s - 1 < BLOCKS and s >= 1:
                    dve_inter(vector, s - 1)
                if s >= 3 and s - 3 < BLOCKS:
                    dve_red(vector, s - 3)
            i = vector.tensor_scalar(
                out=MIOU[:, :], in0=VOUT[:, :], scalar1=1e30, scalar2=-1.0,
                op0=Alu.min, op1=Alu.add,
            )
            tk.mark(i, ("vc", 0))

        @block.scalar
        def _(scalar):
            for s in range(BLOCKS + 2):
                if s >= 1 and s - 1 < BLOCKS:
                    act_mid(scalar, s - 1)
                if s >= 2 and s - 2 < BLOCKS:
                    act_rint(scalar, s - 2)
            tk.wait(scalar, ("vc", 0))
            i = _act_recip(scalar, nc, MIOU[:, :], MIOU[:, :])
            tk.mark(i, ("miou", 0))

        @block.gpsimd
        def _(gpsimd):
            for s in range(BLOCKS + 2):
                if s >= 2 and s - 2 < BLOCKS:
                    pool_v(gpsimd, s - 2)

        @block.sync
        def _(sync):
            tk.wait(sync, ("miou", 0))
            sync.dma_start(out=out_ext[:, :], in_=MIOU[:, :]).then_inc(dma_sem, 16)
            sync.wait_ge(dma_sem, 64)

    return nc


def kernel(anchors: np.ndarray, gt_boxes: np.ndarray) -> np.ndarray:
    global LAST_EXEC_NS
    anchors = np.asarray(anchors, dtype=np.float32)
    gt_boxes = np.asarray(gt_boxes, dtype=np.float32)

    apad = np.zeros((N_PAD, 4), dtype=np.float32)
    apad[:N_ANCHORS] = anchors

    g = gt_boxes.reshape(NPAIR, 4)
    garea = (g[:, 2] - g[:, 0]) * (g[:, 3] - g[:, 1])
    gtrows = np.stack([-g[:, 0], g[:, 2], -g[:, 1], g[:, 3], garea]).astype(np.float32)
    gtrows = np.ascontiguousarray(gtrows)

    in_maps = []
    for c in range(N_CORES):
        sh = apad[c * N_LOC : (c + 1) * N_LOC]
        a3 = sh.reshape(P, BLOCKS, 4)
        ap = np.empty_like(a3)
        ap[:, :, 0] = -a3[:, :, 0]
        ap[:, :, 1] = a3[:, :, 2]
        ap[:, :, 2] = -a3[:, :, 1]
        ap[:, :, 3] = a3[:, :, 3]
        aarea = (a3[:, :, 2] - a3[:, :, 0]) * (a3[:, :, 3] - a3[:, :, 1])
        in_maps.append(
            {
                "anchors_p": np.ascontiguousarray(ap.reshape(P, BLOCKS * 4)),
                "aarea": np.ascontiguousarray(aarea.astype(np.float32)),
                "gtrows": gtrows,
            }
        )

    nc = _build_graph()
    trace = os.environ.get("ANCHOR_TRACE", "0") == "1"
    core_ids = list(range(N_CORES))
    if trace:
        _ensure_axon_ntff_hook()
        _patch_upload_artifacts()
        try:
            res = run_bass_kernel_spmd(nc, in_maps, core_ids=core_ids, trace=True)
        except Exception as e:
            print(f"trace run failed ({type(e).__name__}: {e}); falling back", file=sys.stderr)
            res = run_bass_kernel_spmd(nc, in_maps, core_ids=core_ids, trace=False)
    else:
        res = run_bass_kernel_spmd(nc, in_maps, core_ids=core_ids, trace=False)
    LAST_EXEC_NS = res.exec_time_ns

    out = np.empty((BATCH, N_PAD), dtype=np.float32)
    for c in range(N_CORES):
        o = res.results[c]["out"].reshape(P, BLOCKS, 4)
        out[:, c * N_LOC : (c + 1) * N_LOC] = o.transpose(2, 0, 1).reshape(BATCH, N_LOC)
    return out[:, :N_ANCHORS]



# revision 3
# speedup vs baseline: 1.1446x; 1.1446x over previous
"""AnchorTargetLayer max-IoU kernel for 8 TRN2 NeuronCores (v2, fp16).

max_iou[b, n] = max_g IoU(anchor_n, gt_box[b, g]);
anchors [100000, 4] f32, gt_boxes [4, 64, 4] f32 -> out [4, 100000] f32.

Sharding: anchors split 8 ways (12544/core incl pad), gt replicated, no
collectives. Per-core layout: anchors on SBUF partitions (128/block, 98
blocks), all B*G = 256 (batch, gt) pairs on the free dim, batch-major.

Coordinates are pre-scaled by 1/16 on the host and the GT rows cast to
fp16 (per-anchor scalars stay f32; [P,1] scalar operands are exempt from
DVE perf-mode dtype rules), which keeps every intermediate in fp16 range
and unlocks the DVE 2x/4x perf modes. Measured L2 err of this chain vs
the f64 reference is ~2.7e-3 (budget 2e-2).

Math per (anchor, pair), fp16 intermediates:
  tx   = min(gx2, ax2)                       DVE tensor_scalar (4x)
  sx   = min(-gx1, -ax1) + tx      (= iw)    DVE scalar_tensor_tensor (2x)
  ty, sy likewise in y                       DVE
  int  = relu(sx) * sy            (sy relu deferred to the final clamp:
                                   sy<0 makes int<=0 which can only drag
                                   the max negative, fixed at the end)
  sa   = garea + areaA                       ACT Identity+bias (narrow)
  rs   = 1/sa                                ACT Reciprocal (wide)
  w    = int * rs                            Pool tensor_tensor (wide)
  vout[bt] = max_g w                         DVE tensor_reduce (wide)
Final per anchor: v = relu(vout); iou = v / (1 - v).

Blocks are processed in superblocks of C=7; the narrow per-block ops (4
DVE front ops, 1 ACT sa) run per block, everything else once per
superblock over C*256 elements to amortize the TRN2 read-write bubble.
"""

import os
import sys

import numpy as np

sys.path.insert(0, "/opt/trn_rl_repo")

import concourse.bass as bass
import concourse.mybir as mybir
from concourse.bass_utils import run_bass_kernel_spmd

N_ANCHORS = 100000
BATCH = 4
N_GT = 64
N_CORES = 8

P = 128
BLOCKS = 98
C = 7                       # blocks per superblock
NSB = BLOCKS // C           # 14 superblocks
N_LOC = P * BLOCKS          # 12544
N_PAD = N_LOC * N_CORES     # 100352
NPAIR = BATCH * N_GT        # 256
WIDE = C * NPAIR            # 1792
NB = 2                      # superblock double-buffer depth

F32 = mybir.dt.float32
F16 = mybir.dt.float16
COORD_SCALE = 1.0 / 16.0

LAST_EXEC_NS = None


def _ensure_axon_ntff_hook():
    try:
        import antenv.axon_hooks  # noqa: F401

        return
    except ImportError:
        pass
    import contextlib
    import ctypes
    import types

    import antenv

    m = types.ModuleType("antenv.axon_hooks")
    m._hook = None

    def set_axon_ntff_profile_hook(h):
        m._hook = h

    def get_axon_ntff_profile_hook():
        return m._hook

    m.set_axon_ntff_profile_hook = set_axon_ntff_profile_hook
    m.get_axon_ntff_profile_hook = get_axon_ntff_profile_hook
    sys.modules["antenv.axon_hooks"] = m
    antenv.axon_hooks = m

    so_path = os.environ.get("PJRT_LIBRARY_PATH", "/opt/axon/libaxon_pjrt.so")
    try:
        lib = ctypes.CDLL(so_path)
    except OSError:
        return
    if not hasattr(lib, "axon_start_nrt_profile"):
        return
    lib.axon_start_nrt_profile.argtypes = [
        ctypes.POINTER(ctypes.c_int64),
        ctypes.c_size_t,
    ]
    lib.axon_start_nrt_profile.restype = ctypes.c_int64
    lib.axon_stop_nrt_profile.argtypes = [ctypes.c_char_p]
    lib.axon_stop_nrt_profile.restype = ctypes.c_int64

    @contextlib.contextmanager
    def _hook(output_dir, device_ids):
        import jax

        jax.devices()
        if device_ids:
            ids = (ctypes.c_int64 * len(device_ids))(*device_ids)
            rc = lib.axon_start_nrt_profile(ids, len(device_ids))
        else:
            rc = lib.axon_start_nrt_profile(None, 0)
        if rc != 0:
            raise RuntimeError(f"axon_start_nrt_profile rc={rc}")
        try:
            yield
        finally:
            n = lib.axon_stop_nrt_profile(str(output_dir).encode())
            if n < 0:
                raise RuntimeError(f"axon_stop_nrt_profile rc={n}")

    set_axon_ntff_profile_hook(_hook)


def _patch_upload_artifacts():
    import concourse.bass_utils as bu

    if getattr(bu.upload_artifacts, "_safe", False):
        return
    orig = bu.upload_artifacts

    def safe(tmpdir):
        try:
            return orig(tmpdir)
        except Exception:
            return tmpdir

    safe._safe = True
    bu.upload_artifacts = safe


def _act_recip(scalar_eng, nc, out_ap, in_ap, bias=0.0, scale=1.0):
    """Directly emit Activation(Reciprocal) (the nc.scalar.activation wrapper
    rejects Reciprocal)."""
    ins = [scalar_eng.lower_ap(in_ap)]
    for argv in (bias, scale, 0.0):  # bias, scale, alpha
        ins.append(mybir.ImmediateValue(dtype=F32, value=argv))
    return scalar_eng.add_instruction(
        mybir.InstActivation(
            name=nc.get_next_instruction_name(),
            func=mybir.ActivationFunctionType.Reciprocal,
            ins=ins,
            outs=[scalar_eng.lower_ap(out_ap)],
        )
    )


def _build_graph():
    nc = bass.Bass()
    ASC_ext = nc.declare_dram_parameter("ascal", [P, BLOCKS * 4], F32, isOutput=False)
    AR_ext = nc.declare_dram_parameter("aarea", [P, BLOCKS], F32, isOutput=False)
    GT_ext = nc.declare_dram_parameter("gtrows", [5, NPAIR], F16, isOutput=False)
    out_ext = nc.declare_dram_parameter("out", [P, BLOCKS * 4], F32, isOutput=True)

    Alu = mybir.AluOpType

    with (
        nc.sbuf_tensor("ASC", [P, BLOCKS * 4], F32) as ASC,
        nc.sbuf_tensor("AR", [P, BLOCKS], F32) as AR,
        nc.sbuf_tensor("GTB", [P, 5, NPAIR], F16) as GTB,
        nc.sbuf_tensor("TX", [P, NPAIR], F16) as TX,
        nc.sbuf_tensor("SX", [P, NB, C, NPAIR], F16) as SX,
        nc.sbuf_tensor("SY", [P, NB, C, NPAIR], F16) as SY,
        nc.sbuf_tensor("INT", [P, NB, C, NPAIR], F16) as INT,
        nc.sbuf_tensor("SA", [P, NB, C, NPAIR], F16) as SA,
        nc.sbuf_tensor("RS", [P, NB, C, NPAIR], F16) as RS,
        nc.sbuf_tensor("W", [P, NB, C, NPAIR], F16) as W,
        nc.sbuf_tensor("VOUT", [P, BLOCKS * 4], F32) as VOUT,
        nc.sbuf_tensor("V2", [P, BLOCKS * 4], F32) as V2,
        nc.sbuf_tensor("R1", [P, BLOCKS * 4], F32) as R1,
        nc.sbuf_tensor("MIOU", [P, BLOCKS * 4], F32) as MIOU,
        nc.Block() as block,
        nc.semaphore("dma_sem") as dma_sem,
        nc.semaphore("dve_sem") as dve_sem,
        nc.semaphore("act_sem") as act_sem,
        nc.semaphore("pool_sem") as pool_sem,
    ):
        GX2 = GTB[:, 0, :]
        GX1N = GTB[:, 1, :]
        GY2 = GTB[:, 2, :]
        GY1N = GTB[:, 3, :]
        GAREA = GTB[:, 4, :]

        @block.sync
        def _(sync):
            sync.dma_start(out=ASC[:, :], in_=ASC_ext[:, :]).then_inc(dma_sem, 16)
            sync.dma_start(out=AR[:, :], in_=AR_ext[:, :]).then_inc(dma_sem, 16)
            g_ap = GT_ext[:, :]
            g_b = bass.AP(
                tensor=g_ap.tensor, offset=g_ap.offset, ap=[[0, P]] + list(g_ap.ap)
            )
            sync.dma_start(out=GTB[:, :, :], in_=g_b).then_inc(dma_sem, 16)

        # semaphore targets:
        #   dve_sem: s+1 after inter(s); NSB+1 after v2; NSB+2 after miou
        #   act_sem: s+1 after rs(s);    NSB+1 after r1
        #   pool_sem: s+1 after w(s)
        @block.vector
        def _(vector):
            vector.wait_ge(dma_sem, 48)
            for s in range(NSB):
                sb = s % NB
                for j in range(C):
                    blk = s * C + j
                    ax2 = ASC[:, 4 * blk + 0 : 4 * blk + 1]
                    nax1 = ASC[:, 4 * blk + 1 : 4 * blk + 2]
                    ay2 = ASC[:, 4 * blk + 2 : 4 * blk + 3]
                    nay1 = ASC[:, 4 * blk + 3 : 4 * blk + 4]
                    vector.tensor_scalar(
                        out=TX[:, :], in0=GX2, scalar1=ax2, scalar2=None, op0=Alu.min
                    )
                    vector.scalar_tensor_tensor(
                        out=SX[:, sb, j, :], in0=GX1N, scalar=nax1, in1=TX[:, :],
                        op0=Alu.min, op1=Alu.add,
                    )
                    vector.tensor_scalar(
                        out=TX[:, :], in0=GY2, scalar1=ay2, scalar2=None, op0=Alu.min
                    )
                    vector.scalar_tensor_tensor(
                        out=SY[:, sb, j, :], in0=GY1N, scalar=nay1, in1=TX[:, :],
                        op0=Alu.min, op1=Alu.add,
                    )
                # INT slot reused: pool must have consumed superblock s-NB
                if s >= NB:
                    vector.wait_ge(pool_sem, s - NB + 1)
                vector.scalar_tensor_tensor(
                    out=INT[:, sb, :, :], in0=SX[:, sb, :, :], scalar=0.0,
                    in1=SY[:, sb, :, :], op0=Alu.max, op1=Alu.mult,
                ).then_inc(dve_sem, 1)
                # reduce of the PREVIOUS superblock (software pipelining: pool
                # w(s-1) finished while this superblock's front ran)
                if s >= 1:
                    vector.wait_ge(pool_sem, s)
                    vector.tensor_reduce(
                        out=VOUT[:, (s - 1) * C * 4 : s * C * 4],
                        in_=W[:, (s - 1) % NB, :, :].rearrange(
                            "p c (bt g) -> p (c bt) g", bt=BATCH
                        ),
                        axis=mybir.AxisListType.X,
                        op=Alu.max,
                    )
            vector.wait_ge(pool_sem, NSB)
            vector.tensor_reduce(
                out=VOUT[:, (NSB - 1) * C * 4 : NSB * C * 4],
                in_=W[:, (NSB - 1) % NB, :, :].rearrange(
                    "p c (bt g) -> p (c bt) g", bt=BATCH
                ),
                axis=mybir.AxisListType.X,
                op=Alu.max,
            )
            # final: v = relu(vout); iou = v * (1 / (1 - v))
            vector.tensor_scalar(
                out=V2[:, :], in0=VOUT[:, :], scalar1=0.0, scalar2=None, op0=Alu.max
            ).then_inc(dve_sem, 1)
            vector.wait_ge(act_sem, NSB + 1)
            vector.tensor_tensor(
                out=MIOU[:, :], in0=V2[:, :], in1=R1[:, :], op=Alu.mult
            ).then_inc(dve_sem, 1)

        @block.scalar
        def _(scalar):
            scalar.wait_ge(dma_sem, 48)
            for s in range(NSB):
                sb = s % NB
                if s >= NB:
                    scalar.wait_ge(pool_sem, s - NB + 1)
                for j in range(C):
                    blk = s * C + j
                    scalar.activation(
                        out=SA[:, sb, j, :], in_=GAREA,
                        func=mybir.ActivationFunctionType.Identity,
                        bias=AR[:, blk : blk + 1], scale=1.0,
                    )
                _act_recip(
                    scalar, nc, RS[:, sb, :, :], SA[:, sb, :, :]
                ).then_inc(act_sem, 1)
            scalar.wait_ge(dve_sem, NSB + 1)
            _act_recip(
                scalar, nc, R1[:, :], V2[:, :], bias=1.0, scale=-1.0
            ).then_inc(act_sem, 1)

        @block.gpsimd
        def _(gpsimd):
            for s in range(NSB):
                sb = s % NB
                gpsimd.wait_ge(dve_sem, s + 1)
                gpsimd.wait_ge(act_sem, s + 1)
                gpsimd.tensor_tensor(
                    out=W[:, sb, :, :], in0=INT[:, sb, :, :], in1=RS[:, sb, :, :],
                    op=Alu.mult,
                ).then_inc(pool_sem, 1)

        @block.sync
        def _(sync):
            sync.wait_ge(dve_sem, NSB + 2)
            sync.dma_start(out=out_ext[:, :], in_=MIOU[:, :]).then_inc(dma_sem, 16)
            sync.wait_ge(dma_sem, 64)

    return nc


def kernel(anchors: np.ndarray, gt_boxes: np.ndarray) -> np.ndarray:
    global LAST_EXEC_NS
    anchors = np.asarray(anchors, dtype=np.float32) * COORD_SCALE
    gt_boxes = np.asarray(gt_boxes, dtype=np.float32) * COORD_SCALE

    apad = np.zeros((N_PAD, 4), dtype=np.float32)
    apad[:N_ANCHORS] = anchors

    g = gt_boxes.reshape(NPAIR, 4).astype(np.float32)
    garea = (g[:, 2] - g[:, 0]) * (g[:, 3] - g[:, 1])
    gtrows = np.stack([g[:, 2], -g[:, 0], g[:, 3], -g[:, 1], garea])
    gtrows = np.ascontiguousarray(gtrows.astype(np.float16))

    in_maps = []
    for c in range(N_CORES):
        sh = apad[c * N_LOC : (c + 1) * N_LOC]
        a3 = sh.reshape(P, BLOCKS, 4)
        asc = np.empty_like(a3)
        asc[:, :, 0] = a3[:, :, 2]   # ax2
        asc[:, :, 1] = -a3[:, :, 0]  # -ax1
        asc[:, :, 2] = a3[:, :, 3]   # ay2
        asc[:, :, 3] = -a3[:, :, 1]  # -ay1
        aarea = (a3[:, :, 2] - a3[:, :, 0]) * (a3[:, :, 3] - a3[:, :, 1])
        in_maps.append(
            {
                "ascal": np.ascontiguousarray(asc.reshape(P, BLOCKS * 4)),
                "aarea": np.ascontiguousarray(aarea.astype(np.float32)),
                "gtrows": gtrows,
            }
        )

    nc = _build_graph()
    trace = os.environ.get("ANCHOR_TRACE", "0") == "1"
    core_ids = list(range(N_CORES))
    if trace:
        _ensure_axon_ntff_hook()
        _patch_upload_artifacts()
        try:
            res = run_bass_kernel_spmd(nc, in_maps, core_ids=core_ids, trace=True)
        except Exception as e:
            print(f"trace run failed ({type(e).__name__}: {e}); falling back", file=sys.stderr)
            res = run_bass_kernel_spmd(nc, in_maps, core_ids=core_ids, trace=False)
    else:
        res = run_bass_kernel_spmd(nc, in_maps, core_ids=core_ids, trace=False)
    LAST_EXEC_NS = res.exec_time_ns

    out = np.empty((BATCH, N_PAD), dtype=np.float32)
    for c in range(N_CORES):
        o = res.results[c]["out"].reshape(P, BLOCKS, 4)
        out[:, c * N_LOC : (c + 1) * N_LOC] = o.transpose(2, 0, 1).reshape(BATCH, N_LOC)
    return out[:, :N_ANCHORS]
